# revision 8
# baseline (speedup 1.0000x reference)
"""Trainium2 Bass kernel for a 3rd-order HONU layer.

Math: out[b] = sum_{i<=j<=k} w3[i,j,k] * xb[b,i] * xb[b,j] * xb[b,k]
with xb = [1, x] (129 features), w3 = `weight` in lexicographic
combinations_with_replacement order (366145 entries).

Restructuring (no gathers on device):
  - Pairs (j,k), j<=k, lex order; pair index (j,k) -> Q(j) + (k-j),
    Q(j) = j*129 - j*(j-1)/2.  Total pairs NPAIR = 8385.
  - Dense W2[129, 8385]: W2[i, p(j,k)] = w3[i,j,k] for i<=j else 0.
    Because the weight layout is lexicographic, W2[i, Q(i):] is a
    contiguous copy of weight[off_i : off_i + T(129-i)].
  - out[b] = sum_p (xb[b,j]*xb[b,k]) * U[b,p],  U = xb @ W2.

Sharding (combination axis across 8 cores, SPMD-uniform program):
  - j is assigned round-robin: core c, slot s in [0,17) handles j = 8s+c.
  - Slot s gets a fixed width W_s = 129-8s (core c's true width is
    129-8s-c, the tail is zero-padded in the weights) so the program is
    identical on every core; per-core differences live only in the data.
  - Per-core inputs: W2 column slice in slot layout (rows 0..127 for the
    K=128 matmul over xb feats 0..127 = [1,x_0..x_126]; row 128 for a
    K=1 matmul over xb feat 128 = x_127), and xsh[b,t] = xb[b,t+c]
    (shifted xb) so the on-device slot windows use compile-time offsets.
  - Device: U tiles via PE matmuls into PSUM; one fused DVE op per slot:
    out = (xsh_win * xsh_j) * U_win, accum_out = row-sum -> per-slot acc;
    final reduce + DMA out [256,1] partial; host sums the 8 partials.
"""

import os

import numpy as np

import concourse.bass as bass
import concourse.mybir as mybir
from concourse.tile import TileContext
from concourse.bass_utils import run_bass_kernel_spmd

# ---- problem constants (hardcoded; kernel.py must be self-contained) ----
N = 129                      # features incl. bias column
B = 256                      # batch
N_CORES = 8
NPAIR = N * (N + 1) // 2     # 8385
N_SLOTS = 17
SLOT_W = [N - 8 * s for s in range(N_SLOTS)]           # 129, 121, ..., 1
SLOT_OFF = [0]
for _w in SLOT_W:
    SLOT_OFF.append(SLOT_OFF[-1] + _w)
L = SLOT_OFF[-1]             # 1105 local columns per core
# PSUM tile groups of whole slots; widths 363, 445, 297 (all in [256, 512])
GROUPS = [(0, 3), (3, 8), (8, 17)]

_MM_DT_NAME = os.environ.get("HONU_MM_DT", "float32")
_MM_DT = getattr(mybir.dt, _MM_DT_NAME)
_F32 = mybir.dt.float32

LAST_RESULTS = None          # BassKernelResults of the most recent run


def _np_mm_dtype():
    if _MM_DT_NAME == "bfloat16":
        import ml_dtypes
        return ml_dtypes.bfloat16
    return np.float32


def _build_bass():
    """Raw Bass (no Tile): explicit semaphores, at most ONE sync wait per
    instruction (walrus under the bass2jax/BSP path rejects instructions
    carrying more than one wait command)."""
    nc = bass.Bass()
    # xbt (128x256) and w2a (128xL) are packed into one DRAM tensor /
    # one DMA each, so a single shared DMA semaphore covers all loads.
    mm128_d = nc.dram_tensor("mm128", [128, B + L], _MM_DT, kind="ExternalInput")
    mm1_d = nc.dram_tensor("mm1", [1, B + L], _MM_DT, kind="ExternalInput")
    xsh_d = nc.dram_tensor("xsh", [B, N], _F32, kind="ExternalInput")
    out_d = nc.dram_tensor("out", [B, 1], _F32, kind="ExternalOutput")

    mult = mybir.AluOpType.mult

    with (
        nc.sbuf_tensor("mm128_t", [128, B + L], _MM_DT) as mm128_t,
        nc.sbuf_tensor("mm1_t", [1, B + L], _MM_DT) as mm1_t,
        nc.sbuf_tensor("xsh0_t", [128, N], _F32) as xsh0_t,
        nc.sbuf_tensor("xsh1_t", [128, N], _F32) as xsh1_t,
        nc.sbuf_tensor("scr0_t", [128, L], _F32) as scr0_t,
        nc.sbuf_tensor("scr1_t", [128, L], _F32) as scr1_t,
        nc.sbuf_tensor("acc0_t", [128, N_SLOTS], _F32) as acc0_t,
        nc.sbuf_tensor("acc1_t", [128, N_SLOTS], _F32) as acc1_t,
        nc.sbuf_tensor("o0_t", [128, 1], _F32) as o0_t,
        nc.sbuf_tensor("o1_t", [128, 1], _F32) as o1_t,
        nc.psum_tensor("ps0", [128, 512], _F32) as ps0,
        nc.psum_tensor("ps1", [128, 512], _F32) as ps1,
        nc.psum_tensor("ps2", [128, 512], _F32) as ps2,
        nc.psum_tensor("ps3", [128, 512], _F32) as ps3,
        nc.psum_tensor("ps4", [128, 512], _F32) as ps4,
        nc.psum_tensor("ps5", [128, 512], _F32) as ps5,
        nc.semaphore("dma_sem") as dma_sem,
        nc.semaphore("pe_sem") as pe_sem,
        nc.semaphore("dve_sem") as dve_sem,
        nc.semaphore("stt_sem") as stt_sem,
        nc.Block() as block,
    ):
        xsh_ts = [xsh0_t, xsh1_t]
        scr_ts = [scr0_t, scr1_t]
        acc_ts = [acc0_t, acc1_t]
        o_ts = [o0_t, o1_t]
        psums = [ps0, ps1, ps2, ps3, ps4, ps5]

        @block.sync
        def _(sync):
            sync.dma_start(mm128_t[:], mm128_d[:]).then_inc(dma_sem, 16)
            sync.dma_start(mm1_t[:], mm1_d[:]).then_inc(dma_sem, 16)
            sync.dma_start(xsh0_t[:], xsh_d[0:128, :]).then_inc(dma_sem, 16)
            sync.dma_start(xsh1_t[:], xsh_d[128:256, :]).then_inc(dma_sem, 16)
            sync.wait_ge(dve_sem, 1)
            sync.dma_start(out_d[0:128, :], o0_t[:]).then_inc(dma_sem, 16)
            sync.wait_ge(dve_sem, 2)
            sync.dma_start(out_d[128:256, :], o1_t[:]).then_inc(dma_sem, 16)
            sync.wait_ge(dma_sem, 96)

        @block.tensor
        def _(tensor):
            tensor.wait_ge(dma_sem, 64)  # all four input DMAs complete
            for bb in range(2):
                for gi, (s0, s1) in enumerate(GROUPS):
                    g0, g1 = SLOT_OFF[s0], SLOT_OFF[s1]
                    psv = psums[bb * 3 + gi][:, :g1 - g0]
                    nc.tensor.matmul(
                        psv,
                        lhsT=mm128_t[:, bb * 128:(bb + 1) * 128],
                        rhs=mm128_t[:, B + g0:B + g1],
                        start=True, stop=False,
                    )
                    nc.tensor.matmul(
                        psv,
                        lhsT=mm1_t[:, bb * 128:(bb + 1) * 128],
                        rhs=mm1_t[:, B + g0:B + g1],
                        start=False, stop=True,
                    ).then_inc(pe_sem, 1)

        @block.vector
        def _(vector):
            done = 0
            for bb in range(2):
                for gi, (s0, s1) in enumerate(GROUPS):
                    done += 1
                    vector.wait_ge(pe_sem, done)
                    ps = psums[bb * 3 + gi]
                    g0 = SLOT_OFF[s0]
                    for s in range(s0, s1):
                        w = SLOT_W[s]
                        lo = SLOT_OFF[s]
                        ins = nc.vector.scalar_tensor_tensor(
                            out=scr_ts[bb][:, lo:lo + w],
                            in0=xsh_ts[bb][:, 8 * s:8 * s + w],
                            scalar=xsh_ts[bb][:, 8 * s:8 * s + 1],
                            in1=ps[:, lo - g0:lo - g0 + w],
                            op0=mult,
                            op1=mult,
                            accum_out=acc_ts[bb][:, s:s + 1],
                        )
                        if s == N_SLOTS - 1 or s == s1 - 1 and gi == len(GROUPS) - 1:
                            ins.then_inc(stt_sem, 1)
                # Same-engine RAW: the accumulator writes of the in-flight
                # fused ops must drain before the reduce reads them.
                vector.wait_ge(stt_sem, bb + 1)
                nc.vector.reduce_sum(
                    o_ts[bb][:], acc_ts[bb][:], axis=mybir.AxisListType.X
                ).then_inc(dve_sem, 1)
    return nc


_NC_CACHE = None


def _get_nc():
    global _NC_CACHE
    if _NC_CACHE is None:
        _NC_CACHE = _build_bass()
    return _NC_CACHE


def _host_prep(x, weight):
    """Build per-core input maps from the full inputs."""
    mmdt = _np_mm_dtype()
    xb = np.concatenate([np.ones((B, 1), np.float32), x], axis=1)  # [256,129]

    # Global dense W2 [129, 8385]
    W2 = np.zeros((N, NPAIR), np.float32)
    off = 0
    for i in range(N):
        m = (N - i) * (N - i + 1) // 2
        W2[i, NPAIR - m:] = weight[off:off + m]
        off += m

    def Q(j):
        return j * N - j * (j - 1) // 2

    xbt = np.ascontiguousarray(xb[:, :128].T).astype(mmdt)      # [128, 256]
    xlast = np.ascontiguousarray(xb[:, 128:129].T).astype(mmdt)  # [1, 256]

    in_maps = []
    for c in range(N_CORES):
        W2L = np.zeros((N, L), np.float32)
        for s in range(N_SLOTS):
            j = 8 * s + c
            if j >= N:
                continue
            w = N - j
            W2L[:, SLOT_OFF[s]:SLOT_OFF[s] + w] = W2[:, Q(j):Q(j) + w]
        xsh = np.zeros((B, N), np.float32)
        xsh[:, :N - c] = xb[:, c:]
        mm128 = np.concatenate([xbt, W2L[:128].astype(mmdt)], axis=1)
        mm1 = np.concatenate([xlast, W2L[128:129].astype(mmdt)], axis=1)
        in_maps.append({
            "mm128": np.ascontiguousarray(mm128),
            "mm1": np.ascontiguousarray(mm1),
            "xsh": xsh,
        })
    return in_maps


def kernel(x, weight, comb_idx=None):
    """Full inputs in, full output out. comb_idx is implied by the fixed
    lexicographic layout and is not used."""
    global LAST_RESULTS
    x = np.asarray(x, dtype=np.float32)
    weight = np.asarray(weight, dtype=np.float32)
    in_maps = _host_prep(x, weight)
    nc = _get_nc()
    res = run_bass_kernel_spmd(nc, in_maps, list(range(N_CORES)))
    LAST_RESULTS = res
    out = np.zeros((B, 1), np.float64)
    for r in res.results:
        out += r["out"].astype(np.float64)
    return out.astype(np.float32)
